# revision 5
# baseline (speedup 1.0000x reference)
"""Trainium2 Bass kernel for a 2-layer GRU controller step (batch=1).

Model (PyTorch GRU-cell semantics, gates packed [r, z, n]):
    e  = emb[x]                                  [1, 512]
    h0 = GRUCell(e,  h_in[0]; w_ih0, w_hh0, b_ih0, b_hh0)   H=2048
    h1 = GRUCell(h0, h_in[1]; w_ih1, w_hh1, b_ih1, b_hh1)
    probs = softmax(h1 @ w_head.T + b_head)      [1, 16]
    h_out = stack([h0, h1])                      [2, 1, 2048]

Distribution (8 NeuronCores, tensor-parallel per the gate dimension):
  Each core owns a 256-row slice of every gate (r/z/n) of both layers:
  768 rows of each packed weight matrix.  Weights are pre-transposed on
  the host into [K, 768] "K-major" blobs so each [128, 768] K-slab DMAs
  contiguously and slices directly into 128x128 matmul stationary tiles.
  Matvecs run on the PE array as out[128,1] = W_T_tile.T @ x_chunk with
  x in a "chunked" SBUF layout (partition p, col c) = x[c*128 + p].
  gi and gh for the r/z gates accumulate into the same PSUM columns so
  the gate math starts from i_r+h_r directly.  h0 (and h1) shards are
  AllGather-ed across the 8 cores between layers; the head + softmax is
  computed redundantly on every core after the h1 AllGather.

The memory roofline dominates: ~163 MB of f32 weights / 8 cores
(~20.5 MB/core at ~350 GB/s -> ~60 us) or half that in bf16 mode.
"""

import os

import numpy as np

H = 2048
E = 512
LEN_ACTION = 64
NV = 16
NCORES = 8
SH = H // NCORES          # 256 hidden units per core
R = 3 * SH                # 768 gate rows per core per matrix
MT = R // 128             # 6 m-tiles
KH = H // 128             # 16 k-chunks for hidden-sized contractions
KE = E // 128             # 4 k-chunks for embed-sized contraction

# Weight dtype: "f32" or "bf16" (bf16 halves HBM traffic; ~1e-3 rel err)
WDT = os.environ.get("BASS_GRU_WDT", "bf16")

_CACHE = {}
LAST_RESULTS = None


def _build(wdt_name: str, trace: bool):
    import concourse.bacc as bacc
    import concourse.mybir as mybir
    import concourse.tile as tile

    F32 = mybir.dt.float32
    I32 = mybir.dt.int32
    WD = F32 if wdt_name == "f32" else mybir.dt.bfloat16

    nc = bacc.Bacc("TRN2", target_bir_lowering=False, debug=False,
                   num_devices=NCORES)

    x_idx = nc.dram_tensor("x_idx", [1, 1], I32, kind="ExternalInput")
    emb = nc.dram_tensor("emb", [LEN_ACTION, E], WD, kind="ExternalInput")
    hin_rhs = nc.dram_tensor("hin_rhs", [128, 2 * KH], WD,
                             kind="ExternalInput")
    hp = nc.dram_tensor("hp", [128, 4], F32, kind="ExternalInput")
    biases = nc.dram_tensor("biases", [128, 16], F32, kind="ExternalInput")
    bhead = nc.dram_tensor("bhead", [1, NV], F32, kind="ExternalInput")
    wih0T = nc.dram_tensor("wih0T", [E, R], WD, kind="ExternalInput")
    whh0T = nc.dram_tensor("whh0T", [H, R], WD, kind="ExternalInput")
    wih1T = nc.dram_tensor("wih1T", [H, R], WD, kind="ExternalInput")
    whh1T = nc.dram_tensor("whh1T", [H, R], WD, kind="ExternalInput")
    wheadT = nc.dram_tensor("wheadT", [128, KH * NV], WD,
                            kind="ExternalInput")
    probs_o = nc.dram_tensor("probs", [1, NV], F32, kind="ExternalOutput")
    hout_o = nc.dram_tensor("h_out", [2, H], F32, kind="ExternalOutput")

    with tile.TileContext(nc) as tc:
        with (
            tc.tile_pool(name="wp", bufs=1) as wp,
            tc.tile_pool(name="sb", bufs=1) as sb,
            tc.tile_pool(name="ps", bufs=1, space="PSUM") as ps,
            tc.tile_pool(name="dram", bufs=1, space="DRAM") as dram,
        ):
            # ---------- small input loads ----------
            xs = sb.tile([1, 1], I32, tag="xs")
            nc.sync.dma_start(xs[:], x_idx[:, :])
            embsb = sb.tile([LEN_ACTION, E], WD, tag="embsb")
            nc.sync.dma_start(embsb[:], emb[:, :])
            hin_sb = sb.tile([128, 2 * KH], WD, tag="hin_sb")
            nc.sync.dma_start(hin_sb[:], hin_rhs[:, :])
            hp_sb = sb.tile([128, 4], F32, tag="hp_sb")
            nc.sync.dma_start(hp_sb[:], hp[:, :])
            b_sb = sb.tile([128, 16], F32, tag="b_sb")
            nc.sync.dma_start(b_sb[:], biases[:, :])

            # ---------- e = emb[x] via one-hot matmul ----------
            xf = sb.tile([1, 1], F32, tag="xf")
            nc.vector.tensor_copy(xf[:], xs[:])
            ones = sb.tile([1, 128], F32, tag="ones")
            nc.vector.memset(ones[:], 1.0)
            xbc = ps.tile([128, 1], F32, tag="xbc")
            nc.tensor.matmul(xbc[:], ones[:], xf[:], start=True, stop=True)
            iot = sb.tile([128, 1], F32, tag="iot")
            nc.gpsimd.iota(iot[:], [[0, 1]], channel_multiplier=1,
                           allow_small_or_imprecise_dtypes=True)
            oneh = sb.tile([128, 1], WD, tag="oneh")
            nc.vector.tensor_tensor(oneh[:], iot[:], xbc[:],
                                    mybir.AluOpType.is_equal)
            pse = ps.tile([128, KE], F32, tag="pse")
            for c in range(KE):
                nc.tensor.matmul(pse[:, c:c + 1],
                                 embsb[0:LEN_ACTION, c * 128:(c + 1) * 128],
                                 oneh[0:LEN_ACTION, 0:1],
                                 start=(c == 0), stop=(c == KE - 1),
                                 skip_group_check=True)
            e_sb = sb.tile([128, KE], WD, tag="e_sb")
            nc.vector.tensor_copy(e_sb[:], pse[:])

            # ---------- helpers ----------
            def mm_block(psum, wdram, nk, rhs_fn, col_of, start_fn, stop_fn,
                         name):
                """k-outer loop: DMA one [128, R] K-slab, then 6 matmuls.

                start=True clears has_written for the WHOLE PSUM bank, so
                only the very first matmul into a psum tile may set it;
                start=False overwrites where the bit is clear, which
                bootstraps each column's accumulation correctly."""
                for k in range(nk):
                    slab = wp.tile([128, R], WD, tag=f"{name}_{k}")
                    nc.sync.dma_start(slab[:],
                                      wdram.ap()[k * 128:(k + 1) * 128, :])
                    for m in range(MT):
                        col = col_of(m)
                        nc.tensor.matmul(
                            psum[:, col:col + 1],
                            slab[:, m * 128:(m + 1) * 128],
                            rhs_fn(k),
                            start=start_fn(k, m, col),
                            stop=stop_fn(k, m, col),
                            skip_group_check=True,
                        )

            def gates(psum, l):
                """GRU gate math from psum [128,8]:
                cols 0-3 = i_rz + h_rz, 4-5 = i_n, 6-7 = h_n."""
                bo = l * 8
                rz_b = sb.tile([128, 4], F32, tag=f"rzb{l}")
                nc.vector.tensor_add(rz_b[:], psum[:, 0:4],
                                     b_sb[:, bo:bo + 4])
                rz = sb.tile([128, 4], F32, tag=f"rz{l}")
                nc.scalar.activation(rz[:], rz_b[:],
                                     mybir.ActivationFunctionType.Sigmoid)
                hn_b = sb.tile([128, 2], F32, tag=f"hnb{l}")
                nc.vector.tensor_add(hn_b[:], psum[:, 6:8],
                                     b_sb[:, bo + 6:bo + 8])
                in_b = sb.tile([128, 2], F32, tag=f"inb{l}")
                nc.vector.tensor_add(in_b[:], psum[:, 4:6],
                                     b_sb[:, bo + 4:bo + 6])
                rhn = sb.tile([128, 2], F32, tag=f"rhn{l}")
                nc.vector.tensor_mul(rhn[:], rz[:, 0:2], hn_b[:])
                npre = sb.tile([128, 2], F32, tag=f"npre{l}")
                nc.vector.tensor_add(npre[:], in_b[:], rhn[:])
                n = sb.tile([128, 2], F32, tag=f"n{l}")
                nc.scalar.activation(n[:], npre[:],
                                     mybir.ActivationFunctionType.Tanh)
                d = sb.tile([128, 2], F32, tag=f"d{l}")
                nc.vector.tensor_sub(d[:], hp_sb[:, 2 * l:2 * l + 2], n[:])
                zd = sb.tile([128, 2], F32, tag=f"zd{l}")
                nc.vector.tensor_mul(zd[:], rz[:, 2:4], d[:])
                h = sb.tile([128, 2], F32, tag=f"h{l}")
                nc.vector.tensor_add(h[:], n[:], zd[:])
                return h

            def allgather(h_tile, l):
                ag_in = dram.tile([SH], mybir.dt.float32, tag=f"agi{l}")
                ag_out = dram.tile([H], mybir.dt.float32, tag=f"ago{l}")
                nc.sync.dma_start(ag_in.rearrange("(c p) -> p c", p=128),
                                  h_tile[:])
                nc.gpsimd.collective_compute(
                    "AllGather",
                    mybir.AluOpType.bypass,
                    replica_groups=[list(range(NCORES))],
                    ins=[ag_in[:].opt()],
                    outs=[ag_out[:].opt()],
                )
                hf = sb.tile([128, KH], F32, tag=f"hf{l}")
                nc.sync.dma_start(hf[:],
                                  ag_out.rearrange("(c p) -> p c", p=128))
                # full-state output row l
                nc.sync.dma_start(
                    hout_o.ap()[l, :].rearrange("(c p) -> p c", p=128),
                    hf[:])
                if WD is F32:
                    return hf
                hfw = sb.tile([128, KH], WD, tag=f"hfw{l}")
                nc.vector.tensor_copy(hfw[:], hf[:])
                return hfw

            # ---------- layer 0 ----------
            p0 = ps.tile([128, 8], F32, tag="p0")
            mm_block(p0, wih0T, KE, lambda k: e_sb[:, k:k + 1],
                     col_of=lambda m: m,
                     start_fn=lambda k, m, col: k == 0 and m == 0,
                     stop_fn=lambda k, m, col: False,
                     name="wih0")
            mm_block(p0, whh0T, KH, lambda k: hin_sb[:, k:k + 1],
                     col_of=lambda m: m if m < 4 else m + 2,
                     start_fn=lambda k, m, col: False,
                     stop_fn=lambda k, m, col: k == KH - 1 and m == MT - 1,
                     name="whh0")
            h0 = gates(p0, 0)
            h0f = allgather(h0, 0)

            # ---------- layer 1 (gh first: its rhs is known from t=0) ----
            p1 = ps.tile([128, 8], F32, tag="p1")
            mm_block(p1, whh1T, KH, lambda k: hin_sb[:, KH + k:KH + k + 1],
                     col_of=lambda m: m if m < 4 else m + 2,
                     start_fn=lambda k, m, col: k == 0 and m == 0,
                     stop_fn=lambda k, m, col: False,
                     name="whh1")
            mm_block(p1, wih1T, KH, lambda k: h0f[:, k:k + 1],
                     col_of=lambda m: m,
                     start_fn=lambda k, m, col: False,
                     stop_fn=lambda k, m, col: k == KH - 1 and m == MT - 1,
                     name="wih1")
            h1 = gates(p1, 1)
            h1f = allgather(h1, 1)

            # ---------- head + softmax (redundant on every core) --------
            whead_sb = sb.tile([128, KH * NV], WD, tag="whead_sb")
            nc.sync.dma_start(whead_sb[:], wheadT[:, :])
            bh_sb = sb.tile([1, NV], F32, tag="bh_sb")
            nc.sync.dma_start(bh_sb[:], bhead[:, :])
            psh = ps.tile([1, NV], F32, tag="psh")
            for c in range(KH):
                nc.tensor.matmul(psh[0:1, :], h1f[:, c:c + 1],
                                 whead_sb[:, c * NV:(c + 1) * NV],
                                 start=(c == 0), stop=(c == KH - 1))
            logit = sb.tile([1, NV], F32, tag="logit")
            nc.vector.tensor_add(logit[:], psh[0:1, :], bh_sb[:])
            lmax = sb.tile([1, 1], F32, tag="lmax")
            nc.vector.tensor_reduce(lmax[:], logit[:],
                                    axis=mybir.AxisListType.X,
                                    op=mybir.AluOpType.max)
            shift = sb.tile([1, NV], F32, tag="shift")
            nc.vector.tensor_scalar(shift[:], logit[:], lmax[0:1, 0:1], None,
                                    mybir.AluOpType.subtract)
            ex = sb.tile([1, NV], F32, tag="ex")
            nc.scalar.activation(ex[:], shift[:],
                                 mybir.ActivationFunctionType.Exp)
            ssum = sb.tile([1, 1], F32, tag="ssum")
            nc.vector.tensor_reduce(ssum[:], ex[:],
                                    axis=mybir.AxisListType.X,
                                    op=mybir.AluOpType.add)
            rs = sb.tile([1, 1], F32, tag="rs")
            nc.vector.reciprocal(rs[:], ssum[:])
            prb = sb.tile([1, NV], F32, tag="prb")
            nc.vector.tensor_scalar(prb[:], ex[:], rs[0:1, 0:1], None,
                                    mybir.AluOpType.mult)
            nc.sync.dma_start(probs_o[:, :], prb[:])

    nc.compile()
    return nc


def _chunk(v):
    """[n*128] -> [128, n] chunked layout: out[p, c] = v[c*128 + p]."""
    return np.ascontiguousarray(v.reshape(-1, 128).T)


def _prep_inputs(x, h_in, emb, w_ih0, w_hh0, b_ih0, b_hh0,
                 w_ih1, w_hh1, b_ih1, b_hh1, w_head, b_head, np_wdt):
    """Build the per-core input maps (shard + transpose + cast)."""
    f32 = np.float32
    x_i = np.asarray(x).astype(np.int32).reshape(1, 1)
    h_in = np.asarray(h_in, f32)
    emb = np.ascontiguousarray(np.asarray(emb)).astype(np_wdt)
    w = {0: (np.asarray(w_ih0, f32), np.asarray(w_hh0, f32)),
         1: (np.asarray(w_ih1, f32), np.asarray(w_hh1, f32))}
    b = {0: (np.asarray(b_ih0, f32), np.asarray(b_hh0, f32)),
         1: (np.asarray(b_ih1, f32), np.asarray(b_hh1, f32))}
    w_head = np.asarray(w_head, f32)
    b_head = np.asarray(b_head, f32).reshape(1, NV)

    hin_rhs = np.concatenate([_chunk(h_in[0, 0]), _chunk(h_in[1, 0])],
                             axis=1).astype(np_wdt)
    # wheadT[p, c*16+j] = w_head[j, c*128+p]
    wheadT = np.ascontiguousarray(
        w_head.T.reshape(KH, 128, NV).transpose(1, 0, 2).reshape(128, KH * NV)
    ).astype(np_wdt)

    in_maps = []
    for k in range(NCORES):
        sl = slice(k * SH, (k + 1) * SH)

        def rows(mat):
            return np.concatenate([mat[g * H:(g + 1) * H][sl]
                                   for g in range(3)], axis=0)

        core = {
            "x_idx": x_i, "emb": emb, "hin_rhs": hin_rhs, "wheadT": wheadT,
            "bhead": b_head,
            "hp": np.concatenate([_chunk(h_in[0, 0, sl]),
                                  _chunk(h_in[1, 0, sl])], axis=1),
        }
        for l in (0, 1):
            wih, whh = w[l]
            core[f"wih{l}T"] = np.ascontiguousarray(rows(wih).T).astype(np_wdt)
            core[f"whh{l}T"] = np.ascontiguousarray(rows(whh).T).astype(np_wdt)
        blobs = []
        for l in (0, 1):
            bih, bhh = b[l]
            brz = (bih + bhh)
            blobs += [_chunk(np.concatenate([brz[g * H:(g + 1) * H][sl]
                                             for g in range(2)])),
                      _chunk(bih[2 * H:][sl]), _chunk(bhh[2 * H:][sl])]
        core["biases"] = np.ascontiguousarray(
            np.concatenate(blobs, axis=1), dtype=f32)
        in_maps.append(core)
    return in_maps


def kernel(x, h_in, emb, w_ih0, w_hh0, b_ih0, b_hh0,
           w_ih1, w_hh1, b_ih1, b_hh1, w_head, b_head):
    global LAST_RESULTS
    import ml_dtypes
    from concourse.bass_utils import run_bass_kernel_spmd

    wdt = WDT
    np_wdt = np.float32 if wdt == "f32" else ml_dtypes.bfloat16
    trace = bool(int(os.environ.get("BASS_KERNEL_TRACE", "0")))

    key = (wdt,)
    if key not in _CACHE:
        _CACHE[key] = _build(wdt, trace)
    nc = _CACHE[key]

    in_maps = _prep_inputs(x, h_in, emb, w_ih0, w_hh0, b_ih0, b_hh0,
                           w_ih1, w_hh1, b_ih1, b_hh1, w_head, b_head,
                           np_wdt)
    res = run_bass_kernel_spmd(nc, in_maps, core_ids=list(range(NCORES)),
                               trace=trace)
    LAST_RESULTS = res
    probs = np.asarray(res.results[0]["probs"], np.float32)
    h_out = np.asarray(res.results[0]["h_out"],
                       np.float32).reshape(2, 1, H)
    return probs, h_out
